# revision 13
# baseline (speedup 1.0000x reference)
"""AttentionLSTM Trainium2 kernel: data-parallel over batch on 8 NeuronCores.

Reference semantics (per batch element n):
  A_flat = A.reshape(N, H, 16); h0 = c0 = mean_p(A_flat)
  xWx = x @ Wx
  per step t:
    scores[p] = (h . A_flat[:, p]) / sqrt(H)
    w = softmax(scores); attn = A_flat @ w
    a = xWx_t + h @ Wh + attn @ Wattn + b
    i,f,o,g = sig/sig/sig/tanh of quarters; c = f*c + i*g; h = o*tanh(c)
  out[:, t, :] = h

Shapes: N=512, T=64, D=512, H=512 (4H=2048). 8 cores, 64 batch each.

Kernel mapping per core (n=64 local batch). PE efficiency is the limit at
M=64, so every N=512 matmul is issued as a 2x column-tiled PAIR: the same
[128,64] stationary is loaded into array column halves 0-63 and 64-127
(tile_position (0,0) / (0,64)), and the two concurrent streams carry TWO
DIFFERENT outputs (two E-chunks of the big GEMM, or two halves of the
scores cross-term). Each PSUM accumulation group is a complete sum, so no
combine step is needed; gates for the (0,64)-half chunks are read from
PSUM partitions 64-127 and written to SBUF partitions 0-63 by ScalarE
(cross-partition ACT verified on HW). has_written clears are
per-written-element, so the two column groups may accumulate
independently provided each group's first matmul uses start=True.

  - scores: X[m,(p,n)] pair -> mask-mul (bf16) -> contiguous reduce.
  - softmax kept on the sigmoid ACT table: e^x = sig(x)/(1-sig(x)).
  - attn: block-diagonal matmul, stationary A_PT, moving wBD 8-col blocks.
  - GEMM: chunk pairs (i,f) and (g,o); bias via paired K=1 ones matmuls.
  - gates on ACT from PSUM; c-update chain on GpSimd; hT evacs on ACT.
"""

import math
import sys

sys.path.insert(0, "/opt/trn_rl_repo")

import numpy as np
import ml_dtypes

import concourse.bass as bass
import concourse.mybir as mybir
from concourse.tile import TileContext
from concourse.bass_utils import run_bass_kernel_spmd

N, T, D, H = 512, 64, 512, 512
E = 4 * H  # 2048
NCORES = 8
NL = N // NCORES  # 64 batch per core
P16 = 16  # attention positions
NB = 8  # batch blocks of 8 for block-diag attn
SCALE = 1.0 / math.sqrt(H)

F32 = mybir.dt.float32
MM_DT = mybir.dt.bfloat16

# E-chunk column slices: quarters [i|f|o|g] of the fused weight.
CH_I, CH_F, CH_O, CH_G = 0, 1, 2, 3


def build_nc(reps=1):
    nc = bass.Bass("TRN2", target_bir_lowering=False)

    # --- DRAM I/O ---
    xT_d = nc.declare_dram_parameter("xT", [T, D, NL], MM_DT, isOutput=False)
    AhT_d = nc.declare_dram_parameter("AhT", [H, P16 * NL], MM_DT, isOutput=False)
    APT_d = nc.declare_dram_parameter("APT", [128, NB * H], MM_DT, isOutput=False)
    W_d = nc.declare_dram_parameter("W", [3 * H, E], MM_DT, isOutput=False)
    b_d = nc.declare_dram_parameter("bias", [1, E], MM_DT, isOutput=False)
    h0_d = nc.declare_dram_parameter("h0", [NL, H], F32, isOutput=False)
    h0T_d = nc.declare_dram_parameter("h0T", [H, NL], MM_DT, isOutput=False)
    i64_d = nc.declare_dram_parameter("i64", [NL, NL], F32, isOutput=False)
    d16_d = nc.declare_dram_parameter("d16", [P16, 128], MM_DT, isOutput=False)
    mPN_d = nc.declare_dram_parameter("mPN", [128, 8 * NL], F32, isOutput=False)
    mBD64_d = nc.declare_dram_parameter("mBD64", [128, NL], MM_DT, isOutput=False)
    ones1_d = nc.declare_dram_parameter("ones1", [1, NL], MM_DT, isOutput=False)
    out_d = nc.declare_dram_parameter("out", [NL, T, H], F32, isOutput=True)

    Sig = mybir.ActivationFunctionType.Sigmoid
    Tanh = mybir.ActivationFunctionType.Tanh

    with TileContext(nc) as tc:
        with (
            tc.tile_pool(name="wpool", bufs=1) as wpool,
            tc.tile_pool(name="state", bufs=1) as state,
            tc.tile_pool(name="xin", bufs=3) as xin,
            tc.tile_pool(name="work", bufs=2) as work,
            tc.tile_pool(name="hout", bufs=2) as hout,
            tc.tile_pool(name="psA", bufs=1, space="PSUM") as psA,
            tc.tile_pool(name="psB", bufs=1, space="PSUM") as psB,
        ):
            # ---- persistent SBUF tensors ----
            W_sb = wpool.tile([128, 12, E], MM_DT, tag="W")  # 12 K-tiles of W
            nc.sync.dma_start(
                out=W_sb[:], in_=W_d.ap().rearrange("(k p) e -> p k e", p=128)
            )
            b_sb = wpool.tile([1, E], MM_DT, tag="bias")
            nc.sync.dma_start(out=b_sb[:], in_=b_d[:])
            AhT_sb = wpool.tile([128, 4, P16 * NL], MM_DT, tag="AhT")
            nc.sync.dma_start(
                out=AhT_sb[:], in_=AhT_d.ap().rearrange("(k p) f -> p k f", p=128)
            )
            APT_sb = wpool.tile([128, NB, H], MM_DT, tag="APT")
            nc.sync.dma_start(
                out=APT_sb[:], in_=APT_d.ap().rearrange("p (b h) -> p b h", b=NB)
            )
            i64_sb = wpool.tile([NL, NL], F32, tag="i64")
            nc.sync.dma_start(out=i64_sb[:], in_=i64_d[:])
            d16_sb = wpool.tile([P16, 128], MM_DT, tag="d16")
            nc.sync.dma_start(out=d16_sb[:], in_=d16_d[:])
            mPN_sb = wpool.tile([128, 8 * NL], F32, tag="mPN")
            nc.sync.dma_start(out=mPN_sb[:], in_=mPN_d[:])
            mBD64_sb = wpool.tile([128, NL], MM_DT, tag="mBD64")
            nc.sync.dma_start(out=mBD64_sb[:], in_=mBD64_d[:])
            ones1_sb = wpool.tile([1, NL], MM_DT, tag="ones1")
            nc.sync.dma_start(out=ones1_sb[:], in_=ones1_d[:])

            # state: c (64, 512) and hT as 4 tiles (128, 64)
            c_sb = state.tile([NL, H], F32, tag="c")
            nc.sync.dma_start(out=c_sb[:], in_=h0_d[:])
            hT_sb = state.tile([128, 4, NL], MM_DT, tag="hT")
            nc.sync.dma_start(
                out=hT_sb[:], in_=h0T_d.ap().rearrange("(k p) n -> p k n", p=128)
            )

            # ---- PSUM tiles: 6 banks in psA, 2 in psB ----
            ps_i = psA.tile([128, 512], F32, tag="psi")  # rows 0-63
            ps_f = psA.tile([128, 512], F32, tag="psf")  # rows 64-127
            ps_g = psA.tile([128, 512], F32, tag="psg")  # rows 0-63
            ps_o = psA.tile([128, 512], F32, tag="pso")  # rows 64-127
            X_ps = psA.tile([128, 2, 512], F32, tag="X")  # Xa rows 0-63, Xb 64-127
            at_ps = psB.tile([128, 4, NL], F32, tag="atps")
            small_ps = psB.tile([128, 6 * NL], F32, tag="small")

            def gemm_pair(k, stat, cA, cB, psA_t, psB_t, start, stop):
                sA = slice(cA * 512, (cA + 1) * 512)
                sB = slice(cB * 512, (cB + 1) * 512)
                nc.tensor.matmul(
                    psA_t[0:64, :], stat, W_sb[:, k, sA],
                    start=start, stop=stop, tile_position=(0, 0),
                )
                nc.tensor.matmul(
                    psB_t[64:128, :], stat, W_sb[:, k, sB],
                    start=start, stop=stop, tile_position=(0, 64),
                )

            def bias_pair(cA, cB, psA_t, psB_t):
                sA = slice(cA * 512, (cA + 1) * 512)
                sB = slice(cB * 512, (cB + 1) * 512)
                nc.tensor.matmul(
                    psA_t[0:64, :], ones1_sb[:], b_sb[:, sA],
                    start=False, stop=True, tile_position=(0, 0),
                )
                nc.tensor.matmul(
                    psB_t[64:128, :], ones1_sb[:], b_sb[:, sB],
                    start=False, stop=True, tile_position=(0, 64),
                )

            def scores_pair(j, half):
                """Scores cross-term X[m,(p,n)] for one 512-col half (bank):
                left 256 cols on psum rows 0-63, right 256 on rows 64-127.
                Bank-sequential halves let half-0's diag DVE overlap half-1's
                matmuls without a PSUM bank hazard."""
                base = half * 512
                nc.tensor.matmul(
                    X_ps[0:64, half, 0:256], hT_sb[:, j],
                    AhT_sb[:, j, base : base + 256],
                    start=(j == 0), stop=(j == 3), tile_position=(0, 0),
                )
                nc.tensor.matmul(
                    X_ps[64:128, half, 256:512], hT_sb[:, j],
                    AhT_sb[:, j, base + 256 : base + 512],
                    start=(j == 0), stop=(j == 3), tile_position=(0, 64),
                )

            def bias_pair(cA, cB, psA_t, psB_t):
                """Bias via K=1 ones matmul; opens each bank's accumulation
                group (start=True) so it lands in the cover region."""
                sA = slice(cA * 512, (cA + 1) * 512)
                sB = slice(cB * 512, (cB + 1) * 512)
                nc.tensor.matmul(
                    psA_t[0:64, :], ones1_sb[:], b_sb[:, sA],
                    start=True, stop=False, tile_position=(0, 0),
                )
                nc.tensor.matmul(
                    psB_t[64:128, :], ones1_sb[:], b_sb[:, sB],
                    start=True, stop=False, tile_position=(0, 64),
                )

            def head_if(n_, xt_tile):
                """Always-ready (i,f) filler pairs for the tail: bias then
                xT K-tiles 0..2."""
                if n_ == 0:
                    bias_pair(CH_I, CH_F, ps_i, ps_f)
                else:
                    gemm_pair(n_ - 1, xt_tile[:, n_ - 1], CH_I, CH_F, ps_i, ps_f,
                              start=False, stop=False)

            def head_rest(xt_tile):
                gemm_pair(3, xt_tile[:, 3], CH_I, CH_F, ps_i, ps_f,
                          start=False, stop=False)
                bias_pair(CH_G, CH_O, ps_g, ps_o)
                for k in range(4):
                    gemm_pair(k, xt_tile[:, k], CH_G, CH_O, ps_g, ps_o,
                              start=False, stop=False)

            _lp = tc.For_i(0, reps, 1) if reps > 1 else None
            if _lp is not None:
                _lp.__enter__()

            # prologue: stream x_0, scores for t=0, open t=0's acc groups
            cur_x = xin.tile([128, 4, NL], MM_DT, tag="xT")
            nc.sync.dma_start(
                out=cur_x[:], in_=xT_d[0].rearrange("(k p) n -> p k n", p=128)
            )
            for j in range(4):
                scores_pair(j, 0)
            for j in range(4):
                scores_pair(j, 1)
            for n_ in range(4):
                head_if(n_, cur_x)
            head_rest(cur_x)

            for t in range(T):
                # ---- prefetch x_{t+1}^T early ----
                nxt_x = None
                if t < T - 1:
                    nxt_x = xin.tile([128, 4, NL], MM_DT, tag="xT")
                    nc.sync.dma_start(
                        out=nxt_x[:],
                        in_=xT_d[t + 1].rearrange("(k p) n -> p k n", p=128),
                    )

                # ---- hT-part pairs: the chain's main PE cover ----
                for k in range(4):
                    gemm_pair(4 + k, hT_sb[:, k], CH_I, CH_F, ps_i, ps_f,
                              start=False, stop=False)
                for k in range(4):
                    gemm_pair(4 + k, hT_sb[:, k], CH_G, CH_O, ps_g, ps_o,
                              start=False, stop=False)

                # ---- diagonal extract per half: 2 muls + grouped reduce;
                #      half 0's DVE overlaps half 1's scores matmuls ----
                Xm = work.tile([NL, P16 * NL], F32, tag="Xm")
                scS = work.tile([NL, P16], F32, tag="scS")
                for half in range(2):
                    o = half * 512
                    nc.vector.tensor_mul(
                        Xm[:, o : o + 256],
                        X_ps[0:64, half, 0:256], mPN_sb[0:64, 0:256],
                    )
                    nc.vector.tensor_mul(
                        Xm[:, o + 256 : o + 512],
                        X_ps[64:128, half, 256:512], mPN_sb[64:128, 256:512],
                    )
                    nc.vector.reduce_sum(
                        scS[:, half * 8 : half * 8 + 8],
                        Xm[:, o : o + 512].rearrange("q (p n) -> q p n", p=8),
                        axis=mybir.AxisListType.X,
                    )
                # ---- exp via sigmoid (stays on the sigmoid ACT table) ----
                sg = work.tile([NL, P16], F32, tag="sg")
                nc.scalar.activation(sg[:], scS[:], Sig, scale=SCALE)
                om = work.tile([NL, P16], F32, tag="om")
                nc.vector.tensor_scalar(
                    om[:], sg[:], -1.0, 1.0,
                    op0=mybir.AluOpType.mult, op1=mybir.AluOpType.add,
                )
                omr = work.tile([NL, P16], F32, tag="omr")
                nc.vector.reciprocal(omr[:], om[:])
                expS = work.tile([NL, P16], F32, tag="expS")
                nc.vector.tensor_mul(expS[:], sg[:], omr[:])
                den = work.tile([NL, 1], F32, tag="den")
                nc.vector.reduce_sum(den[:], expS[:], axis=mybir.AxisListType.X)
                rd = work.tile([NL, 1], F32, tag="rd")
                nc.vector.reciprocal(rd[:], den[:])
                wS = work.tile([NL, 2 * P16], MM_DT, tag="wS")
                nc.vector.tensor_scalar_mul(wS[:, 0:P16], expS[:], rd[:])

                # ---- wST via DVE 32x32 block transposes (cols 16-31 junk) ----
                wST = work.tile([2 * P16, NL], MM_DT, tag="wST")
                nc.vector.transpose(wST[:, 0:32], wS[0:32, :])
                nc.vector.transpose(wST[:, 32:64], wS[32:64, :])
                # ---- replicate p-rows x8 and mask -> wBD ----
                rep_ps = small_ps[:, 64:128]
                nc.tensor.matmul(
                    rep_ps, d16_sb[:], wST[0:P16, :], start=True, stop=True
                )
                wBD = work.tile([128, NL], MM_DT, tag="wBD")
                nc.vector.tensor_mul(wBD[:], rep_ps, mBD64_sb[:])

                # ---- attnT: block-diag matmuls, evac per j-tile ----
                attnT = work.tile([128, 4, NL], MM_DT, tag="attnT")
                for j in range(4):
                    for bb in range(NB):
                        nc.tensor.matmul(
                            at_ps[:, j, bb * 8 : (bb + 1) * 8],
                            APT_sb[:, bb, j * 128 : (j + 1) * 128],
                            wBD[:, bb * 8 : (bb + 1) * 8],
                            start=True,
                            stop=True,
                        )
                    nc.vector.tensor_copy(attnT[:, j], at_ps[:, j])

                # ---- (i,f) attn K-tiles (close group); gates i,f ----
                for k in range(8, 12):
                    gemm_pair(k, attnT[:, k - 8], CH_I, CH_F, ps_i, ps_f,
                              start=False, stop=(k == 11))
                ig = work.tile([NL, H], F32, tag="ig")
                fg = work.tile([NL, H], F32, tag="fg")
                nc.scalar.activation(ig[:], ps_i[0:64, :], Sig)
                nc.scalar.activation(fg[:], ps_f[64:128, :], Sig)
                fcp = work.tile([NL, H], F32, tag="fcp")
                nc.gpsimd.tensor_mul(fcp[:], fg[:], c_sb[:])

                # ---- (g,o) attn K-tiles (close group) ----
                for k in range(8, 12):
                    gemm_pair(k, attnT[:, k - 8], CH_G, CH_O, ps_g, ps_o,
                              start=False, stop=(k == 11))

                # ---- sliced tail: gates g,o -> c -> h -> hT -> next scores,
                #      with always-ready (i,f) head pairs as PE filler ----
                gg = work.tile([NL, H], F32, tag="gg")
                og = work.tile([NL, H], F32, tag="og")
                igp = work.tile([NL, H], F32, tag="igp")
                tc_sb = work.tile([NL, H], F32, tag="tc")
                hN = hout.tile([NL, H], F32, tag="hN")
                for j in range(4):
                    js = slice(j * 128, (j + 1) * 128)
                    nc.scalar.activation(gg[:, js], ps_g[0:64, js], Tanh)
                    nc.scalar.activation(og[:, js], ps_o[64:128, js], Sig)
                    nc.vector.tensor_mul(igp[:, js], ig[:, js], gg[:, js])
                    nc.vector.tensor_add(c_sb[:, js], fcp[:, js], igp[:, js])
                    nc.scalar.activation(tc_sb[:, js], c_sb[:, js], Tanh)
                    nc.vector.tensor_mul(hN[:, js], og[:, js], tc_sb[:, js])
                    if t < T - 1:
                        tp_ps = small_ps[:, 128 + 64 * j : 192 + 64 * j]
                        nc.tensor.transpose(tp_ps, hN[:, js], i64_sb[:])
                        nc.scalar.copy(hT_sb[:, j], tp_ps)
                        scores_pair(j, 0)
                        head_if(j, nxt_x)
                if t < T - 1:
                    for j in range(4):
                        scores_pair(j, 1)
                    head_rest(nxt_x)

                # ---- DMA out ----
                nc.sync.dma_start(out=out_d[:, t, :], in_=hN[:])

            if _lp is not None:
                _lp.__exit__(None, None, None)

    _split_matmul_waits(nc)
    return nc


def _split_matmul_waits(nc):
    """Several TPB instruction encodings accept only one sync-wait command;
    hoist excess waits onto an inserted same-engine drain."""
    cnt = 0
    for f in nc.m.functions:
        for blk in f.blocks:
            new_insts = []
            for ins in blk.instructions:
                if (
                    ins.sync_info is not None
                    and ins.sync_info.on_wait
                    and len(ins.sync_info.on_wait) > 1
                ):
                    waits = list(ins.sync_info.on_wait)
                    for w in waits[:-1]:
                        cnt += 1
                        d = mybir.InstDrain(
                            name=f"I-mmw{cnt}", ins=[], outs=[],
                            engine=ins.engine,
                        )
                        d.sync_info = mybir.SyncInfo(on_wait=[w], on_update=[])
                        new_insts.append(d)
                    ins.sync_info = mybir.SyncInfo(
                        on_wait=[waits[-1]], on_update=list(ins.sync_info.on_update or [])
                    )
                new_insts.append(ins)
            blk.instructions = new_insts


def _prep_core_inputs(x_i, A_i, Wx, Wh, Wattn, b):
    """Host-side layout prep for one core's shard (x_i: (64,T,D), A_i: (64,H,4,4))."""
    nl = x_i.shape[0]
    A_flat = A_i.reshape(nl, H, P16)
    h0 = A_flat.mean(axis=2).astype(np.float32)  # (64, H)

    xT = np.ascontiguousarray(x_i.transpose(1, 2, 0)).astype(np.float32)  # (T, D, 64)
    # AhT[h, p*64+n] = A_flat[n, h, p]
    AhT = np.ascontiguousarray(
        A_flat.transpose(1, 2, 0).reshape(H, P16 * nl)
    ).astype(np.float32)
    # APT[(p, n_sub), (b, h)] = A_flat[8b + n_sub, h, p]
    APT = np.ascontiguousarray(
        A_flat.reshape(NB, 8, H, P16).transpose(3, 1, 0, 2).reshape(128, NB * H)
    ).astype(np.float32)
    W = np.concatenate([Wx, Wh, Wattn], axis=0).astype(np.float32)  # (1536, E)
    i64 = np.eye(NL, dtype=np.float32)
    d16 = np.repeat(np.eye(P16, dtype=np.float32), 8, axis=1)  # (16, 128)
    # mPN[m or m+64, p_local*64+n] = (n == m): diag mask for both X halves
    mPN = np.tile(np.tile(np.eye(NL, dtype=np.float32), (1, 8)), (2, 1))  # (128, 512)
    mBD64 = np.tile(np.tile(np.eye(8, dtype=np.float32), (1, 8)), (P16, 1))  # (128,64)
    ones1 = np.ones((1, NL), dtype=np.float32)
    bf16 = ml_dtypes.bfloat16
    return {
        "xT": xT.astype(bf16),
        "AhT": AhT.astype(bf16),
        "APT": APT.astype(bf16),
        "W": W.astype(bf16),
        "bias": b.reshape(1, E).astype(bf16),
        "h0": h0,
        "h0T": np.ascontiguousarray(h0.T).astype(bf16),
        "i64": i64,
        "d16": d16.astype(bf16),
        "mPN": mPN,
        "mBD64": mBD64.astype(bf16),
        "ones1": ones1.astype(bf16),
    }


_NC_CACHE = {}


def kernel(x, A, Wx, Wh, Wattn, b, _trace=False):
    x = np.asarray(x, dtype=np.float32)
    A = np.asarray(A, dtype=np.float32)
    Wx = np.asarray(Wx, dtype=np.float32)
    Wh = np.asarray(Wh, dtype=np.float32)
    Wattn = np.asarray(Wattn, dtype=np.float32)
    b = np.asarray(b, dtype=np.float32)

    if "nc" not in _NC_CACHE:
        _NC_CACHE["nc"] = build_nc()
    nc = _NC_CACHE["nc"]

    in_maps = []
    for i in range(NCORES):
        sl = slice(i * NL, (i + 1) * NL)
        in_maps.append(_prep_core_inputs(x[sl], A[sl], Wx, Wh, Wattn, b))

    res = run_bass_kernel_spmd(
        nc, in_maps, core_ids=list(range(NCORES)), trace=_trace
    )
    outs = [res.results[i]["out"] for i in range(NCORES)]
    full = np.concatenate(outs, axis=0)  # (N, T, H)
    if _trace:
        kernel.last_exec_time_ns = res.exec_time_ns
        kernel.last_profile = res.profile_json
    return full


kernel.last_exec_time_ns = None
kernel.last_profile = None


# revision 15
# speedup vs baseline: 1.2377x; 1.2377x over previous
"""AttentionLSTM Trainium2 kernel: data-parallel over batch on 8 NeuronCores.

Reference semantics (per batch element n):
  A_flat = A.reshape(N, H, 16); h0 = c0 = mean_p(A_flat)
  xWx = x @ Wx
  per step t:
    scores[p] = (h . A_flat[:, p]) / sqrt(H)
    w = softmax(scores); attn = A_flat @ w
    a = xWx_t + h @ Wh + attn @ Wattn + b
    i,f,o,g = sig/sig/sig/tanh of quarters; c = f*c + i*g; h = o*tanh(c)
  out[:, t, :] = h

Shapes: N=512, T=64, D=512, H=512 (4H=2048). 8 cores, 64 batch each.

Kernel mapping per core (n=64 local batch). PE efficiency is the limit at
M=64, so every N=512 matmul is issued as a 2x column-tiled PAIR: the same
[128,64] stationary is loaded into array column halves 0-63 and 64-127
(tile_position (0,0) / (0,64)), and the two concurrent streams carry TWO
DIFFERENT outputs (two E-chunks of the big GEMM, or two halves of the
scores cross-term). Each PSUM accumulation group is a complete sum, so no
combine step is needed; gates for the (0,64)-half chunks are read from
PSUM partitions 64-127 and written to SBUF partitions 0-63 by ScalarE
(cross-partition ACT verified on HW). has_written clears are
per-written-element, so the two column groups may accumulate
independently provided each group's first matmul uses start=True.

  - scores: X[m,(p,n)] pair -> mask-mul (bf16) -> contiguous reduce.
  - softmax kept on the sigmoid ACT table: e^x = sig(x)/(1-sig(x)).
  - attn: block-diagonal matmul, stationary A_PT, moving wBD 8-col blocks.
  - GEMM: chunk pairs (i,f) and (g,o); bias via paired K=1 ones matmuls.
  - gates on ACT from PSUM; c-update chain on GpSimd; hT evacs on ACT.
"""

import math
import sys

sys.path.insert(0, "/opt/trn_rl_repo")

import numpy as np
import ml_dtypes

import concourse.bass as bass
import concourse.mybir as mybir
from concourse.tile import TileContext, add_dep_helper
from concourse.bass_utils import run_bass_kernel_spmd

N, T, D, H = 512, 64, 512, 512
E = 4 * H  # 2048
NCORES = 8
NL = N // NCORES  # 64 batch per core
P16 = 16  # attention positions
NB = 8  # batch blocks of 8 for block-diag attn
SCALE = 1.0 / math.sqrt(H)

F32 = mybir.dt.float32
MM_DT = mybir.dt.bfloat16

# E-chunk column slices: quarters [i|f|o|g] of the fused weight.
CH_I, CH_F, CH_O, CH_G = 0, 1, 2, 3


def build_nc(reps=1):
    nc = bass.Bass("TRN2", target_bir_lowering=False)

    # --- DRAM I/O ---
    xT_d = nc.declare_dram_parameter("xT", [T, D, NL], MM_DT, isOutput=False)
    AhT_d = nc.declare_dram_parameter("AhT", [H, P16 * NL], MM_DT, isOutput=False)
    APT_d = nc.declare_dram_parameter("APT", [128, NB * H], MM_DT, isOutput=False)
    W_d = nc.declare_dram_parameter("W", [3 * H, E], MM_DT, isOutput=False)
    b_d = nc.declare_dram_parameter("bias", [1, E], MM_DT, isOutput=False)
    h0_d = nc.declare_dram_parameter("h0", [NL, H], F32, isOutput=False)
    h0T_d = nc.declare_dram_parameter("h0T", [H, NL], MM_DT, isOutput=False)
    i64_d = nc.declare_dram_parameter("i64", [NL, NL], F32, isOutput=False)
    d16_d = nc.declare_dram_parameter("d16", [P16, 128], MM_DT, isOutput=False)
    mPN_d = nc.declare_dram_parameter("mPN", [128, 8 * NL], F32, isOutput=False)
    mBD64_d = nc.declare_dram_parameter("mBD64", [128, NL], MM_DT, isOutput=False)
    ones1_d = nc.declare_dram_parameter("ones1", [1, NL], MM_DT, isOutput=False)
    out_d = nc.declare_dram_parameter("out", [NL, T, H], F32, isOutput=True)

    Sig = mybir.ActivationFunctionType.Sigmoid
    Tanh = mybir.ActivationFunctionType.Tanh

    with TileContext(nc) as tc:
        with (
            tc.tile_pool(name="wpool", bufs=1) as wpool,
            tc.tile_pool(name="state", bufs=1) as state,
            tc.tile_pool(name="xin", bufs=3) as xin,
            tc.tile_pool(name="work", bufs=2) as work,
            tc.tile_pool(name="hout", bufs=2) as hout,
            tc.tile_pool(name="psA", bufs=1, space="PSUM") as psA,
            tc.tile_pool(name="psB", bufs=1, space="PSUM") as psB,
        ):
            # ---- persistent SBUF tensors ----
            W_sb = wpool.tile([128, 12, E], MM_DT, tag="W")  # 12 K-tiles of W
            nc.sync.dma_start(
                out=W_sb[:], in_=W_d.ap().rearrange("(k p) e -> p k e", p=128)
            )
            b_sb = wpool.tile([1, E], MM_DT, tag="bias")
            nc.sync.dma_start(out=b_sb[:], in_=b_d[:])
            AhT_sb = wpool.tile([128, 4, P16 * NL], MM_DT, tag="AhT")
            nc.sync.dma_start(
                out=AhT_sb[:], in_=AhT_d.ap().rearrange("(k p) f -> p k f", p=128)
            )
            APT_sb = wpool.tile([128, NB, H], MM_DT, tag="APT")
            nc.sync.dma_start(
                out=APT_sb[:], in_=APT_d.ap().rearrange("p (b h) -> p b h", b=NB)
            )
            i64_sb = wpool.tile([NL, NL], F32, tag="i64")
            nc.sync.dma_start(out=i64_sb[:], in_=i64_d[:])
            d16_sb = wpool.tile([P16, 128], MM_DT, tag="d16")
            nc.sync.dma_start(out=d16_sb[:], in_=d16_d[:])
            mPN_sb = wpool.tile([128, 8 * NL], F32, tag="mPN")
            nc.sync.dma_start(out=mPN_sb[:], in_=mPN_d[:])
            mBD64_sb = wpool.tile([128, NL], MM_DT, tag="mBD64")
            nc.sync.dma_start(out=mBD64_sb[:], in_=mBD64_d[:])
            ones1_sb = wpool.tile([1, NL], MM_DT, tag="ones1")
            nc.sync.dma_start(out=ones1_sb[:], in_=ones1_d[:])

            # state: c (64, 512) and hT as 4 tiles (128, 64)
            c_sb = state.tile([NL, H], F32, tag="c")
            nc.sync.dma_start(out=c_sb[:], in_=h0_d[:])
            hT_sb = state.tile([128, 4, NL], MM_DT, tag="hT")
            nc.sync.dma_start(
                out=hT_sb[:], in_=h0T_d.ap().rearrange("(k p) n -> p k n", p=128)
            )

            # ---- PSUM tiles: 6 banks in psA, 2 in psB ----
            ps_i = psA.tile([128, 512], F32, tag="psi")  # rows 0-63
            ps_f = psA.tile([128, 512], F32, tag="psf")  # rows 64-127
            ps_g = psA.tile([128, 512], F32, tag="psg")  # rows 0-63
            ps_o = psA.tile([128, 512], F32, tag="pso")  # rows 64-127
            X_ps = psA.tile([128, 2, 512], F32, tag="X")  # Xa rows 0-63, Xb 64-127
            at_ps = psB.tile([128, 4, NL], F32, tag="atps")
            small_ps = psB.tile([128, 6 * NL], F32, tag="small")

            def gemm_pair(k, stat, cA, cB, psA_t, psB_t, start, stop):
                sA = slice(cA * 512, (cA + 1) * 512)
                sB = slice(cB * 512, (cB + 1) * 512)
                a = nc.tensor.matmul(
                    psA_t[0:64, :], stat, W_sb[:, k, sA],
                    start=start, stop=stop, tile_position=(0, 0),
                )
                b = nc.tensor.matmul(
                    psB_t[64:128, :], stat, W_sb[:, k, sB],
                    start=start, stop=stop, tile_position=(0, 64),
                )
                return a, b

            def gate(pair, dep, why="cover-gate"):
                """Delay an always-ready cover pair until `dep` completes so
                the scheduler cannot drain it before the serial chain that it
                is meant to keep the PE warm through."""
                if dep is None:
                    return
                for mm in pair:
                    add_dep_helper(mm.ins, dep.ins, sync=True, reason=why)

            def bias_pair(cA, cB, psA_t, psB_t):
                sA = slice(cA * 512, (cA + 1) * 512)
                sB = slice(cB * 512, (cB + 1) * 512)
                nc.tensor.matmul(
                    psA_t[0:64, :], ones1_sb[:], b_sb[:, sA],
                    start=False, stop=True, tile_position=(0, 0),
                )
                nc.tensor.matmul(
                    psB_t[64:128, :], ones1_sb[:], b_sb[:, sB],
                    start=False, stop=True, tile_position=(0, 64),
                )

            def scores_pair(j):
                # X[m, (p, n)]: Xa = p 0-7 (rows 0-63), Xb = p 8-15 (rows 64-127)
                a = nc.tensor.matmul(
                    X_ps[0:64, 0, :], hT_sb[:, j], AhT_sb[:, j, 0:512],
                    start=(j == 0), stop=(j == 3), tile_position=(0, 0),
                )
                b = nc.tensor.matmul(
                    X_ps[64:128, 1, :], hT_sb[:, j], AhT_sb[:, j, 512:1024],
                    start=(j == 0), stop=(j == 3), tile_position=(0, 64),
                )
                return a, b

            def bias_pair(cA, cB, psA_t, psB_t):
                """Bias via K=1 ones matmul; opens each bank's accumulation
                group (start=True) so it lands in the cover region."""
                sA = slice(cA * 512, (cA + 1) * 512)
                sB = slice(cB * 512, (cB + 1) * 512)
                a = nc.tensor.matmul(
                    psA_t[0:64, :], ones1_sb[:], b_sb[:, sA],
                    start=True, stop=False, tile_position=(0, 0),
                )
                b = nc.tensor.matmul(
                    psB_t[64:128, :], ones1_sb[:], b_sb[:, sB],
                    start=True, stop=False, tile_position=(0, 64),
                )
                return a, b

            def head_if(n_, xt_tile):
                """Always-ready (i,f) filler pairs for the tail: bias then
                xT K-tiles 0..2."""
                if n_ == 0:
                    return bias_pair(CH_I, CH_F, ps_i, ps_f)
                return gemm_pair(n_ - 1, xt_tile[:, n_ - 1], CH_I, CH_F,
                                 ps_i, ps_f, start=False, stop=False)

            def head_rest(xt_tile):
                gemm_pair(3, xt_tile[:, 3], CH_I, CH_F, ps_i, ps_f,
                          start=False, stop=False)
                bias_pair(CH_G, CH_O, ps_g, ps_o)
                for k in range(4):
                    gemm_pair(k, xt_tile[:, k], CH_G, CH_O, ps_g, ps_o,
                              start=False, stop=False)

            _lp = tc.For_i(0, reps, 1) if reps > 1 else None
            if _lp is not None:
                _lp.__enter__()

            # prologue: stream x_0, scores for t=0, open t=0's acc groups
            cur_x = xin.tile([128, 4, NL], MM_DT, tag="xT")
            nc.sync.dma_start(
                out=cur_x[:], in_=xT_d[0].rearrange("(k p) n -> p k n", p=128)
            )
            for j in range(4):
                scores_pair(j)
            for n_ in range(4):
                head_if(n_, cur_x)
            head_rest(cur_x)

            for t in range(T):
                # ---- prefetch x_{t+1}^T early ----
                nxt_x = None
                if t < T - 1:
                    nxt_x = xin.tile([128, 4, NL], MM_DT, tag="xT")
                    nc.sync.dma_start(
                        out=nxt_x[:],
                        in_=xT_d[t + 1].rearrange("(k p) n -> p k n", p=128),
                    )

                # ---- hT-part pairs: the chain's main PE cover ----
                hpairs = []
                for k in range(4):
                    hpairs.append(
                        gemm_pair(4 + k, hT_sb[:, k], CH_I, CH_F, ps_i, ps_f,
                                  start=False, stop=False))
                for k in range(4):
                    hpairs.append(
                        gemm_pair(4 + k, hT_sb[:, k], CH_G, CH_O, ps_g, ps_o,
                                  start=False, stop=False))

                # ---- diagonal extract: mask-mul + contiguous reduce ----
                Xm = work.tile([NL, P16 * NL], F32, tag="Xm")
                h_m1 = nc.vector.tensor_mul(
                    Xm[:, 0:512], X_ps[0:64, 0, :], mPN_sb[0:64, 0:512]
                )
                h_m2 = nc.vector.tensor_mul(
                    Xm[:, 512:1024], X_ps[64:128, 1, :], mPN_sb[64:128, 0:512]
                )
                scS = work.tile([NL, P16], F32, tag="scS")
                h_red = nc.vector.reduce_sum(
                    scS[:],
                    Xm[:].rearrange("q (p n) -> q p n", p=P16),
                    axis=mybir.AxisListType.X,
                )
                # ---- exp via sigmoid (stays on the sigmoid ACT table) ----
                sg = work.tile([NL, P16], F32, tag="sg")
                h_sg = nc.scalar.activation(sg[:], scS[:], Sig, scale=SCALE)
                om = work.tile([NL, P16], F32, tag="om")
                nc.vector.tensor_scalar(
                    om[:], sg[:], -1.0, 1.0,
                    op0=mybir.AluOpType.mult, op1=mybir.AluOpType.add,
                )
                omr = work.tile([NL, P16], F32, tag="omr")
                h_omr = nc.vector.reciprocal(omr[:], om[:])
                expS = work.tile([NL, P16], F32, tag="expS")
                nc.vector.tensor_mul(expS[:], sg[:], omr[:])
                den = work.tile([NL, 1], F32, tag="den")
                h_den = nc.vector.reduce_sum(
                    den[:], expS[:], axis=mybir.AxisListType.X
                )
                rd = work.tile([NL, 1], F32, tag="rd")
                nc.vector.reciprocal(rd[:], den[:])
                wS = work.tile([NL, 2 * P16], MM_DT, tag="wS")
                h_ws = nc.vector.tensor_scalar_mul(wS[:, 0:P16], expS[:], rd[:])

                # spread the always-ready hT cover across the serial chain so
                # the PE never idles a full HAM MID window
                for pair, dep in zip(
                    hpairs[1:],
                    (h_m1, h_m2, h_red, h_sg, h_omr, h_den, h_ws),
                ):
                    gate(pair, dep)

                # ---- wST via DVE 32x32 block transposes (cols 16-31 junk) ----
                wST = work.tile([2 * P16, NL], MM_DT, tag="wST")
                nc.vector.transpose(wST[:, 0:32], wS[0:32, :])
                nc.vector.transpose(wST[:, 32:64], wS[32:64, :])
                # ---- replicate p-rows x8 and mask -> wBD ----
                rep_ps = small_ps[:, 64:128]
                nc.tensor.matmul(
                    rep_ps, d16_sb[:], wST[0:P16, :], start=True, stop=True
                )
                wBD = work.tile([128, NL], MM_DT, tag="wBD")
                nc.vector.tensor_mul(wBD[:], rep_ps, mBD64_sb[:])

                # ---- attnT: block-diag matmuls, evac per j-tile ----
                attnT = work.tile([128, 4, NL], MM_DT, tag="attnT")
                for j in range(4):
                    for bb in range(NB):
                        nc.tensor.matmul(
                            at_ps[:, j, bb * 8 : (bb + 1) * 8],
                            APT_sb[:, bb, j * 128 : (j + 1) * 128],
                            wBD[:, bb * 8 : (bb + 1) * 8],
                            start=True,
                            stop=True,
                        )
                    nc.vector.tensor_copy(attnT[:, j], at_ps[:, j])

                # ---- (i,f) attn K-tiles (close group); gates i,f ----
                for k in range(8, 12):
                    gemm_pair(k, attnT[:, k - 8], CH_I, CH_F, ps_i, ps_f,
                              start=False, stop=(k == 11))
                ig = work.tile([NL, H], F32, tag="ig")
                fg = work.tile([NL, H], F32, tag="fg")
                nc.scalar.activation(ig[:], ps_i[0:64, :], Sig)
                nc.scalar.activation(fg[:], ps_f[64:128, :], Sig)
                fcp = work.tile([NL, H], F32, tag="fcp")
                nc.vector.tensor_mul(fcp[:], fg[:], c_sb[:])

                # ---- (g,o) attn K-tiles (close group) ----
                for k in range(8, 12):
                    gemm_pair(k, attnT[:, k - 8], CH_G, CH_O, ps_g, ps_o,
                              start=False, stop=(k == 11))

                # ---- sliced tail: gates g,o -> c -> h -> hT -> next scores,
                #      with always-ready (i,f) head pairs as PE filler ----
                gg = work.tile([NL, H], F32, tag="gg")
                og = work.tile([NL, H], F32, tag="og")
                igp = work.tile([NL, H], F32, tag="igp")
                tc_sb = work.tile([NL, H], F32, tag="tc")
                hN = hout.tile([NL, H], F32, tag="hN")
                for j in range(4):
                    js = slice(j * 128, (j + 1) * 128)
                    nc.scalar.activation(gg[:, js], ps_g[0:64, js], Tanh)
                    nc.scalar.activation(og[:, js], ps_o[64:128, js], Sig)
                    nc.vector.tensor_mul(igp[:, js], ig[:, js], gg[:, js])
                    nc.vector.tensor_add(c_sb[:, js], fcp[:, js], igp[:, js])
                    h_tc = nc.scalar.activation(tc_sb[:, js], c_sb[:, js], Tanh)
                    nc.vector.tensor_mul(hN[:, js], og[:, js], tc_sb[:, js])
                    if t < T - 1:
                        tp_ps = small_ps[:, 128 + 64 * j : 192 + 64 * j]
                        nc.tensor.transpose(tp_ps, hN[:, js], i64_sb[:])
                        nc.scalar.copy(hT_sb[:, j], tp_ps)
                        scores_pair(j)
                        gate(head_if(j, nxt_x), h_tc, "tail-cover")
                if t < T - 1:
                    head_rest(nxt_x)

                # ---- DMA out ----
                nc.sync.dma_start(out=out_d[:, t, :], in_=hN[:])

            if _lp is not None:
                _lp.__exit__(None, None, None)

    _split_matmul_waits(nc)
    return nc


def _split_matmul_waits(nc):
    """Several TPB instruction encodings accept only one sync-wait command;
    hoist excess waits onto an inserted same-engine drain."""
    cnt = 0
    for f in nc.m.functions:
        for blk in f.blocks:
            new_insts = []
            for ins in blk.instructions:
                if (
                    ins.sync_info is not None
                    and ins.sync_info.on_wait
                    and len(ins.sync_info.on_wait) > 1
                ):
                    waits = list(ins.sync_info.on_wait)
                    for w in waits[:-1]:
                        cnt += 1
                        d = mybir.InstDrain(
                            name=f"I-mmw{cnt}", ins=[], outs=[],
                            engine=ins.engine,
                        )
                        d.sync_info = mybir.SyncInfo(on_wait=[w], on_update=[])
                        new_insts.append(d)
                    ins.sync_info = mybir.SyncInfo(
                        on_wait=[waits[-1]], on_update=list(ins.sync_info.on_update or [])
                    )
                new_insts.append(ins)
            blk.instructions = new_insts


def _prep_core_inputs(x_i, A_i, Wx, Wh, Wattn, b):
    """Host-side layout prep for one core's shard (x_i: (64,T,D), A_i: (64,H,4,4))."""
    nl = x_i.shape[0]
    A_flat = A_i.reshape(nl, H, P16)
    h0 = A_flat.mean(axis=2).astype(np.float32)  # (64, H)

    xT = np.ascontiguousarray(x_i.transpose(1, 2, 0)).astype(np.float32)  # (T, D, 64)
    # AhT[h, p*64+n] = A_flat[n, h, p]
    AhT = np.ascontiguousarray(
        A_flat.transpose(1, 2, 0).reshape(H, P16 * nl)
    ).astype(np.float32)
    # APT[(p, n_sub), (b, h)] = A_flat[8b + n_sub, h, p]
    APT = np.ascontiguousarray(
        A_flat.reshape(NB, 8, H, P16).transpose(3, 1, 0, 2).reshape(128, NB * H)
    ).astype(np.float32)
    W = np.concatenate([Wx, Wh, Wattn], axis=0).astype(np.float32)  # (1536, E)
    i64 = np.eye(NL, dtype=np.float32)
    d16 = np.repeat(np.eye(P16, dtype=np.float32), 8, axis=1)  # (16, 128)
    # mPN[m or m+64, p_local*64+n] = (n == m): diag mask for both X halves
    mPN = np.tile(np.tile(np.eye(NL, dtype=np.float32), (1, 8)), (2, 1))  # (128, 512)
    mBD64 = np.tile(np.tile(np.eye(8, dtype=np.float32), (1, 8)), (P16, 1))  # (128,64)
    ones1 = np.ones((1, NL), dtype=np.float32)
    bf16 = ml_dtypes.bfloat16
    return {
        "xT": xT.astype(bf16),
        "AhT": AhT.astype(bf16),
        "APT": APT.astype(bf16),
        "W": W.astype(bf16),
        "bias": b.reshape(1, E).astype(bf16),
        "h0": h0,
        "h0T": np.ascontiguousarray(h0.T).astype(bf16),
        "i64": i64,
        "d16": d16.astype(bf16),
        "mPN": mPN,
        "mBD64": mBD64.astype(bf16),
        "ones1": ones1.astype(bf16),
    }


_NC_CACHE = {}


def kernel(x, A, Wx, Wh, Wattn, b, _trace=False):
    x = np.asarray(x, dtype=np.float32)
    A = np.asarray(A, dtype=np.float32)
    Wx = np.asarray(Wx, dtype=np.float32)
    Wh = np.asarray(Wh, dtype=np.float32)
    Wattn = np.asarray(Wattn, dtype=np.float32)
    b = np.asarray(b, dtype=np.float32)

    if "nc" not in _NC_CACHE:
        _NC_CACHE["nc"] = build_nc()
    nc = _NC_CACHE["nc"]

    in_maps = []
    for i in range(NCORES):
        sl = slice(i * NL, (i + 1) * NL)
        in_maps.append(_prep_core_inputs(x[sl], A[sl], Wx, Wh, Wattn, b))

    res = run_bass_kernel_spmd(
        nc, in_maps, core_ids=list(range(NCORES)), trace=_trace
    )
    outs = [res.results[i]["out"] for i in range(NCORES)]
    full = np.concatenate(outs, axis=0)  # (N, T, H)
    if _trace:
        kernel.last_exec_time_ns = res.exec_time_ns
        kernel.last_profile = res.profile_json
    return full


kernel.last_exec_time_ns = None
kernel.last_profile = None
